# revision 1
# baseline (speedup 1.0000x reference)
"""BERT self-attention (BS=4, SEQ=2048, HID=768, NH=12) on 8 NeuronCores.

Sharding: core c -> batch b = c//2, head-group g = c%2 (6 heads each).
Per core the Bass kernel computes, for its batch element and 6 heads:
  Q^T/K^T = (Wh @ X^T + b)   in [d, q] layout  (d on partitions)
  V       = X @ Wv^T + bv    in [k, d] layout, rows scaled by mask m[k]
  S^T     = K^T.T-free matmul -> [k_block, q] scores in PSUM
  P^T     = exp(S^T / 8)     (ACT, PSUM->SBUF; mask folded into V)
  ctx^T   = V'.T @ P^T accumulated over k blocks, with a 65th row = mask
            column giving the softmax denominator.
  out     = ctx^T[0:64] * broadcast(1/denom)  -> [64, q] per head
Host does input transposes (free), sharding, and the final [d,q]->[q,d]
untranspose + concat.

Biases are folded in via an appended ones-row on X^T (contraction 769).
"""

import numpy as np

import concourse.bass as bass
import concourse.tile as tile
from concourse import bacc
from concourse import mybir
from concourse.bass_utils import run_bass_kernel_spmd

F32 = mybir.dt.float32
F32R = mybir.dt.float32r
F16 = mybir.dt.float16
DT_MM = F16          # dtype for matmul operands (A/B: F16 vs F32R)
DT_NP = np.float16   # matching numpy dtype for host-side input prep

BS, SEQ, HID, NH, HD = 4, 2048, 768, 12, 64
NCORES = 8
HPC = 6          # heads per core
FCH = 6          # 128-row chunks of the 768 contraction dim
DSH = HPC * HD   # 384 output features per core


def _body(tc, xt_d, wq_d, wk_d, wv_d, mt_d, ot_d):
    nc = tc.nc
    Exp = mybir.ActivationFunctionType.Exp

    with tc.tile_pool(name="persist", bufs=1) as persist:
        # Warm the exp table set ASAP (overlaps the input DMAs).
        dummy = persist.tile([1, 1], F32, tag="dummy")
        nc.vector.memset(dummy, 0.0)
        nc.scalar.activation(out=dummy, in_=dummy, func=Exp)

        mtile = persist.tile([128, 16], DT_MM, tag="mtile")
        nc.sync.dma_start(out=mtile, in_=mt_d[:, :])
        mtf = persist.tile([128, 16], F32, tag="mtf")
        nc.vector.tensor_copy(out=mtf, in_=mtile)
        qt = [persist.tile([128, SEQ], DT_MM, tag=f"qt{j}", name=f"qt{j}") for j in range(3)]
        kt = [persist.tile([128, SEQ], DT_MM, tag=f"kt{j}", name=f"kt{j}") for j in range(3)]
        vt = persist.tile([128, 16, DSH], DT_MM, tag="vt")

        # ---------------- Phase 1: QKV projections ----------------
        with tc.tile_pool(name="xw", bufs=1) as xw, \
             tc.tile_pool(name="qkp", bufs=3, space="PSUM") as qkp, \
             tc.tile_pool(name="vp", bufs=2, space="PSUM") as vp:
            xts = []
            for f in range(FCH):
                t = xw.tile([128, SEQ], DT_MM, tag=f"x{f}")
                nc.sync.dma_start(out=t, in_=xt_d[f * 128:(f + 1) * 128, :])
                xts.append(t)
            xt1 = persist.tile([1, SEQ], DT_MM, tag="x6")
            nc.sync.dma_start(out=xt1, in_=xt_d[768:769, :])

            wmap = {}
            for dram, nm in ((wq_d, "q"), (wk_d, "k"), (wv_d, "v")):
                lst = []
                for f in range(FCH):
                    t = xw.tile([128, DSH], DT_MM, tag=f"w{nm}{f}")
                    nc.sync.dma_start(out=t, in_=dram[f * 128:(f + 1) * 128, :])
                    lst.append(t)
                b = xw.tile([1, DSH], DT_MM, tag=f"w{nm}b")
                nc.sync.dma_start(out=b, in_=dram[768:769, :])
                lst.append(b)
                wmap[nm] = lst

            # Q^T, K^T: [384, 2048] as 3 tiles of [128, 2048]
            for nm, dst in (("q", qt), ("k", kt)):
                wt = wmap[nm]
                for j in range(3):
                    js = slice(j * 128, (j + 1) * 128)
                    for qc in range(4):
                        qs = slice(qc * 512, (qc + 1) * 512)
                        ps = qkp.tile([128, 512], F32, tag="qk")
                        for f in range(FCH):
                            nc.tensor.matmul(ps, lhsT=wt[f][:, js],
                                             rhs=xts[f][:, qs],
                                             start=(f == 0), stop=False)
                        nc.tensor.matmul(ps, lhsT=wt[6][:, js],
                                         rhs=xt1[:, qs],
                                         start=False, stop=True)
                        nc.scalar.copy(out=dst[j][:, qs], in_=ps)

            # V: [2048, 384] as 16 k-blocks, mask-scaled rows
            wt = wmap["v"]
            for kb in range(16):
                ks = slice(kb * 128, (kb + 1) * 128)
                ps = vp.tile([128, DSH], F32, tag="v")
                for f in range(FCH):
                    nc.tensor.matmul(ps, lhsT=xts[f][:, ks], rhs=wt[f],
                                     start=(f == 0), stop=False)
                nc.tensor.matmul(ps, lhsT=xt1[:, ks], rhs=wt[6],
                                 start=False, stop=True)
                nc.vector.tensor_scalar_mul(
                    out=vt[:, kb, :], in0=ps,
                    scalar1=mtf[:, kb:kb + 1])

        # ---------------- Phase 2: attention ----------------
        # PSUM: scores 2x[128,1024] (4 banks) + ctx pair [128,1024] (2) +
        # denoms [97,1024] (2) = 8 banks.
        with tc.tile_pool(name="sp", bufs=2, space="PSUM") as sp, \
             tc.tile_pool(name="cp", bufs=1, space="PSUM") as cp, \
             tc.tile_pool(name="dp", bufs=1, space="PSUM") as dp, \
             tc.tile_pool(name="pp", bufs=3) as pp, \
             tc.tile_pool(name="ctp", bufs=4) as ctp, \
             tc.tile_pool(name="rdp", bufs=4) as rdp, \
             tc.tile_pool(name="osp", bufs=3) as osp:
            for j in range(3):
                heads = (2 * j, 2 * j + 1)
                ostage = {h: osp.tile([64, SEQ], F32, tag="os", name=f"os{h}")
                          for h in heads}
                for qh in range(2):
                    q0 = qh * 1024
                    cab = cp.tile([128, 1024], F32, tag="c", name="cab")
                    dn = dp.tile([97, 1024], F32, tag="d", name="dn")
                    for kb in range(16):
                        ks = slice(kb * 128, (kb + 1) * 128)
                        sab = [sp.tile([128, 1024], F32, tag="s", name="sab")
                               for _ in range(2)]
                        # scores: 2-head row-packed pairs (K=64 at rows 0/64)
                        for qq in range(2):
                            qs = slice(q0 + qq * 512, q0 + (qq + 1) * 512)
                            osl = slice(qq * 512, (qq + 1) * 512)
                            for i in range(2):
                                rows = slice(64 * i, 64 * (i + 1))
                                nc.tensor.matmul(sab[i][:, osl],
                                                 lhsT=kt[j][rows, ks],
                                                 rhs=qt[j][rows, qs],
                                                 start=True, stop=True)
                        pab = []
                        for i in range(2):
                            p = pp.tile([128, 1024], DT_MM, tag="p", name="ptile")
                            nc.scalar.activation(out=p, in_=sab[i], func=Exp,
                                                 scale=0.125)
                            pab.append(p)
                        st, sp_ = (kb == 0), (kb == 15)
                        # ctx: col-packed pair (head A -> out rows 0-63,
                        # head B -> rows 64-127 of the same PSUM tile)
                        for qq in range(2):
                            osl = slice(qq * 512, (qq + 1) * 512)
                            for i in range(2):
                                nc.tensor.matmul(
                                    cab[64 * i:64 * (i + 1), osl],
                                    lhsT=vt[:, kb, heads[i] * 64:(heads[i] + 1) * 64],
                                    rhs=pab[i][:, osl], start=st, stop=sp_,
                                    skip_group_check=True)
                        # denominators: 4-way col-packed m=1 matmuls
                        # rows 0/32 = heads A/B cols 0:512; 64/96 = cols 512:1024
                        for idx, (i, qq) in enumerate(((0, 0), (1, 0), (0, 1), (1, 1))):
                            osl = slice(qq * 512, (qq + 1) * 512)
                            r = 32 * idx
                            nc.tensor.matmul(dn[r:r + 1, osl],
                                             lhsT=mtile[:, kb:kb + 1],
                                             rhs=pab[i][:, osl],
                                             start=st, stop=sp_,
                                             tile_position=(0, r),
                                             skip_group_check=True)
                    # drain: out = ctx / denom
                    for i in range(2):
                        h = heads[i]
                        ct = ctp.tile([64, 1024], F32, tag="ct")
                        nc.vector.tensor_copy(out=ct, in_=cab[64 * i:64 * (i + 1), :])
                        rd = rdp.tile([1, 1024], DT_MM, tag="rd")
                        nc.vector.tensor_copy(out=rd[:, 0:512],
                                              in_=dn[32 * i:32 * i + 1, 0:512])
                        nc.vector.tensor_copy(out=rd[:, 512:1024],
                                              in_=dn[64 + 32 * i:64 + 32 * i + 1, 512:1024])
                        bc = sp.tile([64, 1024], F32, tag="s")
                        for qq in range(2):
                            osl = slice(qq * 512, (qq + 1) * 512)
                            nc.tensor.matmul(bc[:, osl], lhsT=xt1[:, 0:64],
                                             rhs=rd[:, osl],
                                             start=True, stop=True)
                        rcp = ctp.tile([64, 1024], F32, tag="rcp")
                        nc.vector.reciprocal(rcp, bc)
                        nc.vector.tensor_mul(out=ostage[h][:, q0:q0 + 1024],
                                             in0=ct, in1=rcp)
                for h in heads:
                    nc.sync.dma_start(out=ot_d[h], in_=ostage[h])


def build_nc():
    nc = bacc.Bacc("TRN2")
    xt_d = nc.declare_dram_parameter("xt", [HID + 1, SEQ], DT_MM, isOutput=False)
    wq_d = nc.declare_dram_parameter("wqT", [HID + 1, DSH], DT_MM, isOutput=False)
    wk_d = nc.declare_dram_parameter("wkT", [HID + 1, DSH], DT_MM, isOutput=False)
    wv_d = nc.declare_dram_parameter("wvT", [HID + 1, DSH], DT_MM, isOutput=False)
    mt_d = nc.declare_dram_parameter("mt", [128, 16], DT_MM, isOutput=False)
    ot_d = nc.declare_dram_parameter("OT", [HPC, HD, SEQ], F32, isOutput=True)
    with tile.TileContext(nc) as tc:
        _body(tc, xt_d, wq_d, wk_d, wv_d, mt_d, ot_d)
    nc.finalize()
    return nc


_NC_CACHE = None


def _get_nc():
    global _NC_CACHE
    if _NC_CACHE is None:
        _NC_CACHE = build_nc()
    return _NC_CACHE


def make_in_maps(hidden_states, attention_mask, Wq, bq, Wk, bk, Wv, bv):
    in_maps = []
    for c in range(NCORES):
        b, g = c // 2, c % 2
        hs = slice(g * DSH, (g + 1) * DSH)
        xt = np.empty((HID + 1, SEQ), DT_NP)
        xt[:HID] = hidden_states[b].T
        xt[HID] = 1.0
        m = (attention_mask[b, 0, 0] > -1).astype(DT_NP)
        mt = np.ascontiguousarray(m.reshape(16, 128).T)

        def aug(W, bias):
            wa = np.empty((HID + 1, DSH), DT_NP)
            wa[:HID] = W[hs, :].T
            wa[HID] = bias[hs]
            return wa

        in_maps.append({
            "xt": np.ascontiguousarray(xt),
            "wqT": aug(Wq, bq),
            "wkT": aug(Wk, bk),
            "wvT": aug(Wv, bv),
            "mt": mt,
        })
    return in_maps


def gather_out(results):
    out = np.empty((BS, SEQ, HID), np.float32)
    for c in range(NCORES):
        b, g = c // 2, c % 2
        ot = results[c]["OT"]  # [6, 64, 2048]
        out[b, :, g * DSH:(g + 1) * DSH] = (
            ot.transpose(2, 0, 1).reshape(SEQ, DSH)
        )
    return out


def kernel(hidden_states, attention_mask, Wq, bq, Wk, bk, Wv, bv):
    nc = _get_nc()
    in_maps = make_in_maps(hidden_states, attention_mask,
                           Wq, bq, Wk, bk, Wv, bv)
    res = run_bass_kernel_spmd(nc, in_maps, core_ids=list(range(NCORES)))
    return gather_out(res.results)



# revision 11
# speedup vs baseline: 1.7036x; 1.7036x over previous
"""BERT self-attention (BS=4, SEQ=2048, HID=768, NH=12) on 8 NeuronCores.

Sharding: core c -> batch b = c//2, head-group g = c%2 (6 heads each).

v2 design (vs v1 baseline at 573us):
  * Softmax denominator comes free from the ctx matmul: V is stored per
    head as 65 columns (64 V dims + the 0/1 mask column), so ctx PSUM
    row 64 accumulates sum_k m_k * P[k,q].  This removes the 384 M=1
    denominator matmuls (196K PE columns, ~27% of v1's PE work).
  * Scores for the head pair are packed side by side in one
    [128k, 2*512q] PSUM tile -> ONE exp per kb iteration.
  * The PE is kept continuously busy (pstate ramp to 2.4 GHz) by
    interleaving the QKV projection matmuls as filler inside the
    attention sweeps; Q^T chunks for (j,qc) are produced just in time.
  * QK PSUM drains moved off the ACT engine (saturated by exp) to DVE,
    fused with the bias add (tensor_scalar add) -> no bias matmuls.
  * 1/denom via DVE reciprocal_approx_fast (~5x faster than
    reciprocal), broadcast to 64 rows with a tiny ones-matmul.

Per (j, qc, kb) inner iteration:
  S:   2 matmuls  [64d,128k]x[64d,512q] -> sab[:, i*512:]   (1024 cols)
  exp: 1 ACT      [128,1024] PSUM -> pab f16 SBUF
  C:   2 matmuls  [128k,65]x[128k,512q] -> ctx_i [65,512]   (1024 cols)
Biases are folded in: Q/K via the DVE drain, V via an appended
ones-row on X^T (contraction 769).
"""

import numpy as np

import concourse.bass as bass
import concourse.tile as tile
from concourse import bacc
from concourse import mybir
from concourse.bass_utils import run_bass_kernel_spmd

F32 = mybir.dt.float32
F16 = mybir.dt.float16
DT_MM = F16
DT_NP = np.float16

BS, SEQ, HID, NH, HD = 4, 2048, 768, 12, 64
NCORES = 8
HPC = 6          # heads per core
FCH = 6          # 128-row chunks of the 768 contraction dim
DSH = HPC * HD   # 384 output features per core
QC = 4           # q chunks of 512
KB = 16          # k blocks of 128


def _filler_schedule():
    """(j, qc) -> list of ('q'|'k', jj, qq) projection chunks to compute
    as PE filler during that attention sweep.  Deadlines: qt[j][:,qc]
    before sweep (j,qc); kt[j] fully before sweep (j,0)."""
    sched = {
        (0, 0): [("q", 0, 1)],                        # + V JIT lives here
        (0, 1): [("q", 0, 2), ("k", 1, 0), ("k", 1, 1)],
        (0, 2): [("q", 0, 3), ("k", 1, 2), ("k", 1, 3)],
        (0, 3): [("q", 1, 0)],
        (1, 0): [("q", 1, 1), ("k", 2, 0)],
        (1, 1): [("q", 1, 2), ("k", 2, 1), ("k", 2, 2)],
        (1, 2): [("q", 1, 3), ("k", 2, 3)],
        (1, 3): [("q", 2, 0)],
        (2, 0): [("q", 2, 1)],
        (2, 1): [("q", 2, 2)],
        (2, 2): [("q", 2, 3)],
        (2, 3): [],
    }
    return sched


def _body(tc, xt_d, wq_d, wk_d, wv_d, mt_d, qkb_d, ot_d):
    nc = tc.nc
    Exp = mybir.ActivationFunctionType.Exp

    with tc.tile_pool(name="persist", bufs=1) as persist:
        # Warm the exp table ASAP (overlaps the input DMAs).
        dummy = persist.tile([1, 1], F32, tag="dummy")
        nc.vector.memset(dummy, 0.0)
        nc.scalar.activation(out=dummy, in_=dummy, func=Exp)

        mtile = persist.tile([128, KB], DT_MM, tag="mtile")
        nc.sync.dma_start(out=mtile, in_=mt_d[:, :])
        mtf = persist.tile([128, KB], F32, tag="mtf")
        nc.vector.tensor_copy(out=mtf, in_=mtile)
        qkb = persist.tile([128, 6], F32, tag="qkb")
        nc.sync.dma_start(out=qkb, in_=qkb_d[:, :])

        # Input / weight tiles (persist: used throughout for filler work).
        xts, wqs, wks, wvs = [], [], [], []
        for f in range(FCH):
            xts.append(persist.tile([128, SEQ], DT_MM, tag=f"x{f}", name=f"x{f}"))
            wqs.append(persist.tile([128, DSH], DT_MM, tag=f"wq{f}", name=f"wq{f}"))
            wks.append(persist.tile([128, DSH], DT_MM, tag=f"wk{f}", name=f"wk{f}"))
            wvs.append(persist.tile([128, DSH], DT_MM, tag=f"wv{f}", name=f"wv{f}"))
        # First QK group (kt j0) needs all x and wk chunks: stream those first.
        for f in range(FCH):
            nc.sync.dma_start(out=xts[f], in_=xt_d[f * 128:(f + 1) * 128, :])
            nc.sync.dma_start(out=wks[f], in_=wk_d[f * 128:(f + 1) * 128, :])
        for f in range(FCH):
            nc.sync.dma_start(out=wqs[f], in_=wq_d[f * 128:(f + 1) * 128, :])
        for f in range(FCH):
            nc.sync.dma_start(out=wvs[f], in_=wv_d[f * 128:(f + 1) * 128, :])
        wvb = persist.tile([1, DSH], DT_MM, tag="wvb")
        nc.sync.dma_start(out=wvb, in_=wv_d[768:769, :])
        xt1 = persist.tile([1, SEQ], DT_MM, tag="x6")
        nc.sync.dma_start(out=xt1, in_=xt_d[768:769, :])

        # Q^T/K^T [384, 2048] per head pair j as [128, 2048] tiles.
        qt = [persist.tile([128, SEQ], DT_MM, tag=f"qt{j}", name=f"qt{j}")
              for j in range(3)]
        kt = [persist.tile([128, SEQ], DT_MM, tag=f"kt{j}", name=f"kt{j}")
              for j in range(3)]
        # V with per-head mask column: [k=128, kb, head, 64 V dims + m].
        vt = persist.tile([128, KB, HPC, HD + 1], DT_MM, tag="vt")
        for h in range(HPC):
            nc.vector.tensor_copy(out=vt[:, :, h, HD], in_=mtile)
        ostage = [persist.tile([64, SEQ], F32, tag=f"os{h}", name=f"os{h}")
                  for h in range(HPC)]

        with tc.tile_pool(name="sp", bufs=2, space="PSUM") as sp, \
             tc.tile_pool(name="cp", bufs=3, space="PSUM") as cp, \
             tc.tile_pool(name="fp", bufs=1, space="PSUM") as fp, \
             tc.tile_pool(name="pp", bufs=3) as pp, \
             tc.tile_pool(name="rdp", bufs=2) as rdp:

            def proj_chunk(kind, j, qc):
                """Q or K projection chunk -> qt/kt[j][:, qc*512:...],
                bias folded into the DVE drain."""
                ps = fp.tile([128, 512], F32, tag="f", name="fq")
                qs = slice(qc * 512, (qc + 1) * 512)
                ws = wqs if kind == "q" else wks
                for f in range(FCH):
                    nc.tensor.matmul(ps, lhsT=ws[f][:, j * 128:(j + 1) * 128],
                                     rhs=xts[f][:, qs],
                                     start=(f == 0), stop=(f == FCH - 1))
                dst = (qt if kind == "q" else kt)[j]
                bcol = j if kind == "q" else 3 + j
                nc.vector.tensor_scalar_add(out=dst[:, qs], in0=ps,
                                            scalar1=qkb[:, bcol:bcol + 1])

            def v_chunk(kb):
                """V k-block kb -> vt[:, kb, :, 0:64], mask-scaled rows.
                Bias via the ones-row (contraction 769)."""
                ps = fp.tile([128, DSH], F32, tag="f", name="fv")
                ks = slice(kb * 128, (kb + 1) * 128)
                for f in range(FCH):
                    nc.tensor.matmul(ps, lhsT=xts[f][:, ks], rhs=wvs[f],
                                     start=(f == 0), stop=False)
                nc.tensor.matmul(ps, lhsT=xt1[:, ks], rhs=wvb,
                                 start=False, stop=True)
                nc.vector.tensor_scalar_mul(
                    out=vt[:, kb, :, 0:HD], in0=ps,
                    scalar1=mtf[:, kb:kb + 1])

            # Prologue: kt j0 (all 4 chunks), qt j0 chunk 0, V blocks 0-1.
            for qc in range(QC):
                proj_chunk("k", 0, qc)
            proj_chunk("q", 0, 0)
            v_chunk(0)
            v_chunk(1)

            sched = _filler_schedule()
            for j in range(3):
                heads = (2 * j, 2 * j + 1)
                for qc in range(QC):
                    qs = slice(qc * 512, (qc + 1) * 512)
                    fillers = list(sched[(j, qc)])
                    # spread fillers over the sweep
                    fill_at = {4 + 4 * i: t for i, t in enumerate(fillers)}
                    ctx = [cp.tile([HD + 1, 512], F32, tag="c", name=f"ctx{i}")
                           for i in range(2)]
                    pabs = [None] * KB
                    for kb in range(KB):
                        ks = slice(kb * 128, (kb + 1) * 128)
                        sab = sp.tile([128, 1024], F32, tag="s", name="sab")
                        for i in range(2):
                            rows = slice(64 * i, 64 * (i + 1))
                            nc.tensor.matmul(sab[:, 512 * i:512 * (i + 1)],
                                             lhsT=kt[j][rows, ks],
                                             rhs=qt[j][rows, qs],
                                             start=True, stop=True,
                                             skip_group_check=True)
                        # trailing ctx of the previous iteration keeps the
                        # PE from idling on exp(kb)
                        if kb >= 1:
                            p = pabs[kb - 1]
                            for i in range(2):
                                nc.tensor.matmul(
                                    ctx[i],
                                    lhsT=vt[:, kb - 1, heads[i], :],
                                    rhs=p[:, 512 * i:512 * (i + 1)],
                                    start=(kb - 1 == 0), stop=False)
                        if j == 0 and qc == 0 and kb < KB - 2:
                            v_chunk(kb + 2)
                        if kb in fill_at:
                            proj_chunk(*fill_at[kb])
                        p = pp.tile([128, 1024], DT_MM, tag="p", name="ptile")
                        nc.scalar.activation(out=p, in_=sab, func=Exp,
                                             scale=0.125)
                        pabs[kb] = p
                    for i in range(2):
                        nc.tensor.matmul(ctx[i],
                                         lhsT=vt[:, KB - 1, heads[i], :],
                                         rhs=pabs[KB - 1][:, 512 * i:512 * (i + 1)],
                                         start=False, stop=True)
                    # drain: out = ctx[0:64] * broadcast(1/ctx[64]).
                    # reciprocal_approx_fast is a custom DVE op that drops
                    # the partition offset of its input, so stage the
                    # denominator row to partition 0 in SBUF first.
                    for i in range(2):
                        h = heads[i]
                        dn = rdp.tile([1, 512], F32, tag="dn", name="dn")
                        nc.vector.tensor_copy(out=dn, in_=ctx[i][HD:HD + 1, :])
                        r32 = rdp.tile([1, 512], F32, tag="r32", name="r32")
                        nc.vector.reciprocal_approx_fast(out=r32, in_=dn)
                        rd = rdp.tile([1, 512], DT_MM, tag="r16", name="rd")
                        nc.vector.tensor_copy(out=rd, in_=r32)
                        bc = cp.tile([64, 512], F32, tag="c", name="bc")
                        nc.tensor.matmul(bc, lhsT=xt1[:, 0:64], rhs=rd,
                                         start=True, stop=True)
                        # DVE reads at most one non-scalar PSUM input:
                        # stage the broadcast through SBUF.
                        bcs = rdp.tile([64, 512], F32, tag="bcs", name="bcs")
                        nc.vector.tensor_copy(out=bcs, in_=bc)
                        nc.vector.tensor_mul(out=ostage[h][:, qs],
                                             in0=ctx[i][0:HD, :], in1=bcs)
                    if qc == QC - 1:
                        for h in heads:
                            nc.sync.dma_start(out=ot_d[h], in_=ostage[h])


def build_nc():
    nc = bacc.Bacc("TRN2")
    xt_d = nc.declare_dram_parameter("xt", [HID + 1, SEQ], DT_MM, isOutput=False)
    wq_d = nc.declare_dram_parameter("wqT", [HID, DSH], DT_MM, isOutput=False)
    wk_d = nc.declare_dram_parameter("wkT", [HID, DSH], DT_MM, isOutput=False)
    wv_d = nc.declare_dram_parameter("wvT", [HID + 1, DSH], DT_MM, isOutput=False)
    mt_d = nc.declare_dram_parameter("mt", [128, KB], DT_MM, isOutput=False)
    qkb_d = nc.declare_dram_parameter("qkb", [128, 6], F32, isOutput=False)
    ot_d = nc.declare_dram_parameter("OT", [HPC, HD, SEQ], F32, isOutput=True)
    with tile.TileContext(nc) as tc:
        _body(tc, xt_d, wq_d, wk_d, wv_d, mt_d, qkb_d, ot_d)
    nc.finalize()
    return nc


_NC_CACHE = None


def _get_nc():
    global _NC_CACHE
    if _NC_CACHE is None:
        _NC_CACHE = build_nc()
    return _NC_CACHE


def make_in_maps(hidden_states, attention_mask, Wq, bq, Wk, bk, Wv, bv):
    in_maps = []
    for c in range(NCORES):
        b, g = c // 2, c % 2
        hs = slice(g * DSH, (g + 1) * DSH)
        xt = np.empty((HID + 1, SEQ), DT_NP)
        xt[:HID] = hidden_states[b].T
        xt[HID] = 1.0
        m = (attention_mask[b, 0, 0] > -1).astype(DT_NP)
        mt = np.ascontiguousarray(m.reshape(KB, 128).T)

        qkb = np.empty((128, 6), np.float32)
        for j in range(3):
            qkb[:, j] = bq[g * DSH + j * 128: g * DSH + (j + 1) * 128]
            qkb[:, 3 + j] = bk[g * DSH + j * 128: g * DSH + (j + 1) * 128]

        def augv(W, bias):
            wa = np.empty((HID + 1, DSH), DT_NP)
            wa[:HID] = W[hs, :].T
            wa[HID] = bias[hs]
            return wa

        in_maps.append({
            "xt": np.ascontiguousarray(xt),
            "wqT": np.ascontiguousarray(Wq[hs, :].T.astype(DT_NP)),
            "wkT": np.ascontiguousarray(Wk[hs, :].T.astype(DT_NP)),
            "wvT": augv(Wv, bv),
            "mt": mt,
            "qkb": qkb,
        })
    return in_maps


def gather_out(results):
    out = np.empty((BS, SEQ, HID), np.float32)
    for c in range(NCORES):
        b, g = c // 2, c % 2
        ot = results[c]["OT"]  # [6, 64, 2048]
        out[b, :, g * DSH:(g + 1) * DSH] = (
            ot.transpose(2, 0, 1).reshape(SEQ, DSH)
        )
    return out


def kernel(hidden_states, attention_mask, Wq, bq, Wk, bk, Wv, bv):
    nc = _get_nc()
    in_maps = make_in_maps(hidden_states, attention_mask,
                           Wq, bq, Wk, bk, Wv, bv)
    res = run_bass_kernel_spmd(nc, in_maps, core_ids=list(range(NCORES)))
    return gather_out(res.results)
